# revision 4
# baseline (speedup 1.0000x reference)
"""Multi-head attention (B=8, N=1024, D=1024, H=16) on 8 TRN2 NeuronCores.

Sharding: data-parallel over batch — core i computes batch item i end-to-end.
No collectives. Per-core pipeline (all matmuls in float32r = full PE rate):

  A)  transpose x [N,D] -> xT [D,N]            (PE transpose via identity)
  B1) V = x @ w_v                 (lhsT=xT tiles, rhs=w_v) -> V natural [k,d]
      stored with a ones column per head: V'[k, 65] = [V_h | 1]
  B2) qkT = w_qk^T @ x^T          (lhsT=w_qk tiles, rhs=xT) -> qT,kT [64,N]/head
  C)  per head h: S^T[k,q] = kT^T qT  (K=64, row-packed pairs via base partition)
      expS = exp(0.125*S^T)  (ACT, PSUM->SBUF, f32r out)
      O'^T[65, q] = sum_k V'_h^T expS  -> rows 0..63 = O^T, row 64 = softmax sums
      normalize: O^T * (1/sums)  broadcast along partitions
  D)  out = Onorm^T^T @ w_proj + b  (lhsT=Onorm tiles, rhs=w_proj; bias via
      rank-1 ones x b matmul into the same PSUM accumulation)

softmax max-subtraction is skipped: scores are ~N(0, 0.33^2) here, bounded
well inside exp's fp32 range, so exp/sum is exact enough (matches reference
mathematically).
"""

import os
import sys
import types

sys.path.insert(0, "/opt/trn_rl_repo")

# The agent image's antenv lacks axon_hooks; register the NTFF profile hook
# shim so run_bass_kernel_spmd(trace=True) can capture exec_time_ns.
if "antenv.axon_hooks" not in sys.modules:
    _hooks = types.ModuleType("antenv.axon_hooks")
    _hook_store = [None]
    _hooks.set_axon_ntff_profile_hook = lambda h: _hook_store.__setitem__(0, h)
    _hooks.get_axon_ntff_profile_hook = lambda: _hook_store[0]
    sys.modules["antenv.axon_hooks"] = _hooks
    try:
        from trn_agent_boot.trn_boot import _ntff_profile_via_ctypes

        _hooks.set_axon_ntff_profile_hook(
            _ntff_profile_via_ctypes("/opt/axon/libaxon_pjrt.so")
        )
    except Exception:
        pass

import numpy as np
import concourse.bass as bass
import concourse.bacc as bacc
import concourse.mybir as mybir
import concourse.tile as tile
from concourse import masks
from concourse.bass_utils import run_bass_kernel_spmd

F32 = mybir.dt.float32
F32R = mybir.dt.float32r
EXP = mybir.ActivationFunctionType.Exp

B = 8
N = 1024  # sequence length
D = 1024  # embed dim
H = 16  # heads
HD = 64  # head dim
SCALE = HD**-0.5  # 0.125
NT = N // 128  # 8 seq tiles
DT = D // 128  # 8 embed tiles
NC2 = N // 512  # 2 free-dim chunks of 512

LAST_EXEC_NS = [None]


def build():
    nc = bacc.Bacc(None, target_bir_lowering=False)
    x = nc.declare_dram_parameter("x", [N, D], F32, isOutput=False)
    w_qkv = nc.declare_dram_parameter("w_qkv", [D, 3 * D], F32, isOutput=False)
    w_proj = nc.declare_dram_parameter("w_proj", [D, D], F32, isOutput=False)
    b_proj = nc.declare_dram_parameter("b_proj", [D], F32, isOutput=False)
    out = nc.declare_dram_parameter("out", [N, D], F32, isOutput=True)

    with tile.TileContext(nc) as tc:
        # ---- whole-kernel pools --------------------------------------
        with (
            tc.tile_pool(name="const", bufs=1) as cpool,
            tc.tile_pool(name="xT", bufs=DT) as xTpool,
            tc.tile_pool(name="V", bufs=NT) as Vpool,
            tc.tile_pool(name="qkT", bufs=4) as qkTpool,
            tc.tile_pool(name="Onorm", bufs=DT) as Opool,
            tc.tile_pool(name="es", bufs=4) as espool,
        ):
            ident = cpool.tile([128, 128], F32, tag="ident")
            masks.make_identity(nc, ident[:])
            ones1f = cpool.tile([1, 128], F32, tag="ones1f")
            nc.vector.memset(ones1f[:], 1.0)
            ones1 = cpool.tile([1, 128], F32R, tag="ones1")
            nc.vector.tensor_copy(ones1[:], ones1f[:])
            onescf = cpool.tile([128, H], F32, tag="onescf")
            nc.vector.memset(onescf[:], 1.0)
            b_sb = cpool.tile([1, D], F32R, tag="b_sb")
            nc.sync.dma_start(
                b_sb[:], b_proj[:].rearrange("(a n) -> a n", a=1).bitcast(F32R)
            )

            xT = [xTpool.tile([128, N], F32R, tag="xT", name=f"xT{j}") for j in range(DT)]
            V = [Vpool.tile([128, H * (HD + 1)], F32R, tag="V", name=f"V{i}") for i in range(NT)]
            Onorm = [Opool.tile([128, N], F32R, tag="On", name=f"On{i}") for i in range(DT)]

            # ---- phase A: load + transpose x; phase B1: V projection --
            with (
                tc.tile_pool(name="xin", bufs=3) as xpool,
                tc.tile_pool(name="wv", bufs=DT) as wvpool,
                tc.tile_pool(name="tp", bufs=2, space="PSUM") as tppool,
                tc.tile_pool(name="vp", bufs=3, space="PSUM") as vppool,
            ):
                # w_v tiles: w_qkv[j*128:(j+1)*128, 2048:3072] -> [128, 1024]
                wv = []
                for j in range(DT):
                    t = wvpool.tile([128, D], F32R, tag="wv")
                    nc.sync.dma_start(
                        t[:],
                        w_qkv[j * 128 : (j + 1) * 128, 2 * D : 3 * D].bitcast(F32R),
                    )
                    wv.append(t)

                for i in range(NT):
                    xt = xpool.tile([128, D], F32, tag="x")
                    nc.sync.dma_start(xt[:], x[i * 128 : (i + 1) * 128, :])
                    for j in range(DT):
                        tp = tppool.tile([128, 128], F32, tag="tp")
                        nc.tensor.transpose(
                            tp[:], xt[:, j * 128 : (j + 1) * 128], ident[:]
                        )
                        nc.vector.tensor_copy(
                            xT[j][:, i * 128 : (i + 1) * 128], tp[:]
                        )

                # ones columns of V' (col 64 of each 65-wide head group)
                for i in range(NT):
                    ones_view = V[i][:].rearrange("p (h e) -> p h e", e=HD + 1)
                    nc.vector.tensor_copy(
                        ones_view[:, :, HD : HD + 1],
                        onescf[:].rearrange("p (h e) -> p h e", e=1),
                    )

                # V projection: V[n, e] = sum_d x[n, d] w_v[d, e]
                #   lhsT = xT[j][:, n-slice] (d on partitions), rhs = wv[j]
                for i in range(NT):
                    for c in range(NC2):
                        ps = vppool.tile([128, 512], F32, tag="vp")
                        for j in range(DT):
                            nc.tensor.matmul(
                                ps[:],
                                xT[j][:, i * 128 : (i + 1) * 128],
                                wv[j][:, c * 512 : (c + 1) * 512],
                                start=(j == 0),
                                stop=(j == DT - 1),
                            )
                        # scatter heads into 65-wide groups of V'
                        dst = V[i][:].rearrange("p (h e) -> p h e", e=HD + 1)
                        nc.vector.tensor_copy(
                            dst[:, 8 * c : 8 * c + 8, 0:HD],
                            ps[:].rearrange("p (h e) -> p h e", e=HD),
                        )

            # ---- phases B2 + C: per head-pair qk projection + attention
            with (
                tc.tile_pool(name="wqk", bufs=3) as wqkpool,
                tc.tile_pool(name="rec", bufs=4) as rpool,
                tc.tile_pool(name="recb", bufs=2) as rbpool,
                tc.tile_pool(name="qkp", bufs=2, space="PSUM") as qkppool,
                tc.tile_pool(name="s", bufs=2, space="PSUM") as spool,
                tc.tile_pool(name="o", bufs=2, space="PSUM") as opool,
            ):
                for pair in range(H // 2):
                    # qk projection for this pair: e-tiles `pair` (q rows)
                    # and `8+pair` (k rows) of qkT[e, n] = w_qk^T x^T
                    qk_tiles = []
                    for et in (pair, DT + pair):
                        wq = wqkpool.tile([128, D], F32R, tag="wqk")
                        # gather w_qkv[:, et*128:(et+1)*128] into [128, 8, 128]:
                        # partition = d within j-block, free = (j, e-col)
                        src = w_qkv[:, et * 128 : (et + 1) * 128].rearrange(
                            "(j p) e -> p j e", p=128
                        )
                        nc.sync.dma_start(
                            wq[:].rearrange("p (j e) -> p j e", e=128),
                            src.bitcast(F32R),
                        )
                        qt = qkTpool.tile([128, N], F32R, tag="qkT")
                        for c in range(NC2):
                            ps = qkppool.tile([128, 512], F32, tag="qkp")
                            for j in range(DT):
                                nc.tensor.matmul(
                                    ps[:],
                                    wq[:, j * 128 : (j + 1) * 128],
                                    xT[j][:, c * 512 : (c + 1) * 512],
                                    start=(j == 0),
                                    stop=(j == DT - 1),
                                )
                            nc.vector.tensor_copy(
                                qt[:, c * 512 : (c + 1) * 512], ps[:]
                            )
                        qk_tiles.append(qt)
                    qt_pair, kt_pair = qk_tiles

                    for parity in range(2):
                        h = 2 * pair + parity
                        p0 = 64 * parity
                        qT = qt_pair[p0 : p0 + 64, :]
                        kT = kt_pair[p0 : p0 + 64, :]
                        po = [opool.tile([128, 512], F32, tag="o", name=f"po{c}") for c in range(NC2)]
                        for j in range(NT):
                            ps = spool.tile([128, N], F32, tag="s")
                            for c in range(NC2):
                                nc.tensor.matmul(
                                    ps[:, c * 512 : (c + 1) * 512],
                                    kT[:, j * 128 : (j + 1) * 128],
                                    qT[:, c * 512 : (c + 1) * 512],
                                    start=True,
                                    stop=True,
                                )
                            es = espool.tile([128, N], F32R, tag="es")
                            nc.scalar.activation(es[:], ps[:], EXP, scale=SCALE)
                            for c in range(NC2):
                                nc.tensor.matmul(
                                    po[c][0 : HD + 1, :],
                                    V[j][:, h * (HD + 1) : (h + 1) * (HD + 1)],
                                    es[:, c * 512 : (c + 1) * 512],
                                    start=(j == 0),
                                    stop=(j == NT - 1),
                                )
                        for c in range(NC2):
                            rec = rpool.tile([1, 512], F32, tag="rec")
                            nc.vector.reciprocal(rec[:], po[c][HD : HD + 1, :])
                            recb = rbpool.tile([HD, 512], F32, tag="recb")
                            nc.gpsimd.partition_broadcast(recb[:], rec[:])
                            nc.vector.tensor_mul(
                                Onorm[pair][p0 : p0 + 64, c * 512 : (c + 1) * 512],
                                po[c][0:HD, :],
                                recb[:],
                            )

            # ---- phase D: output projection + bias ----------------------
            with (
                tc.tile_pool(name="wp", bufs=DT) as wppool,
                tc.tile_pool(name="osb", bufs=3) as osbpool,
                tc.tile_pool(name="dp", bufs=3, space="PSUM") as dppool,
            ):
                wp = []
                for ct in range(DT):
                    t = wppool.tile([128, D], F32R, tag="wp")
                    nc.sync.dma_start(
                        t[:], w_proj[ct * 128 : (ct + 1) * 128, :].bitcast(F32R)
                    )
                    wp.append(t)
                for i in range(NT):
                    ot = osbpool.tile([128, D], F32, tag="osb")
                    for ec in range(NC2):
                        ps = dppool.tile([128, 512], F32, tag="dp")
                        for ct in range(DT):
                            nc.tensor.matmul(
                                ps[:],
                                Onorm[ct][:, i * 128 : (i + 1) * 128],
                                wp[ct][:, ec * 512 : (ec + 1) * 512],
                                start=(ct == 0),
                                stop=False,
                            )
                        nc.tensor.matmul(
                            ps[:],
                            ones1[:],
                            b_sb[:, ec * 512 : (ec + 1) * 512],
                            start=False,
                            stop=True,
                        )
                        nc.vector.tensor_copy(
                            ot[:, ec * 512 : (ec + 1) * 512], ps[:]
                        )
                    nc.sync.dma_start(out[i * 128 : (i + 1) * 128, :], ot[:])

    nc.compile()
    return nc


_NC = [None]


def _get_nc():
    if _NC[0] is None:
        _NC[0] = build()
    return _NC[0]


def kernel(x, w_qkv, w_proj, b_proj):
    x = np.asarray(x, dtype=np.float32)
    w_qkv = np.asarray(w_qkv, dtype=np.float32)
    w_proj = np.asarray(w_proj, dtype=np.float32)
    b_proj = np.asarray(b_proj, dtype=np.float32)
    assert x.shape == (B, N, D)

    nc = _get_nc()
    in_maps = [
        {"x": x[i], "w_qkv": w_qkv, "w_proj": w_proj, "b_proj": b_proj}
        for i in range(B)
    ]
    trace = os.environ.get("KERNEL_TRACE") == "1"
    res = run_bass_kernel_spmd(
        nc, in_maps, core_ids=list(range(B)), trace=trace
    )
    LAST_EXEC_NS[0] = res.exec_time_ns
    return np.stack([res.results[i]["out"] for i in range(B)], axis=0)


# revision 23
# speedup vs baseline: 1.8486x; 1.8486x over previous
"""Multi-head attention (B=8, N=1024, D=1024, H=16) on 8 TRN2 NeuronCores.

Sharding: data-parallel over batch — core i computes batch item i end-to-end.
No collectives. Per-core pipeline (all matmuls in float32r = full PE rate):

  A)  transpose x [N,D] -> xT [D,N]            (PE transpose via identity)
  B1) V = x @ w_v                 (lhsT=xT tiles, rhs=w_v) -> V natural [k,d]
      stored with a ones column per head: V'[k, 65] = [V_h | 1]
  B2) qkT = w_qk^T @ x^T          (lhsT=w_qk tiles, rhs=xT) -> qT,kT [64,N]/head
  C)  per head h: S^T[k,q] = kT^T qT  (K=64, row-packed pairs via base partition)
      expS = exp(0.125*S^T)  (ACT, PSUM->SBUF, f32r out)
      O'^T[65, q] = sum_k V'_h^T expS  -> rows 0..63 = O^T, row 64 = softmax sums
      normalize: O^T * (1/sums)  broadcast along partitions
  D)  out = Onorm^T^T @ w_proj + b  (lhsT=Onorm tiles, rhs=w_proj; bias via
      rank-1 ones x b matmul into the same PSUM accumulation)

softmax max-subtraction is skipped: scores are ~N(0, 0.33^2) here, bounded
well inside exp's fp32 range, so exp/sum is exact enough (matches reference
mathematically).
"""

import os
import sys
import types

sys.path.insert(0, "/opt/trn_rl_repo")

# The agent image's antenv lacks axon_hooks; register the NTFF profile hook
# shim so run_bass_kernel_spmd(trace=True) can capture exec_time_ns.
if "antenv.axon_hooks" not in sys.modules:
    _hooks = types.ModuleType("antenv.axon_hooks")
    _hook_store = [None]
    _hooks.set_axon_ntff_profile_hook = lambda h: _hook_store.__setitem__(0, h)
    _hooks.get_axon_ntff_profile_hook = lambda: _hook_store[0]
    sys.modules["antenv.axon_hooks"] = _hooks
    try:
        from trn_agent_boot.trn_boot import _ntff_profile_via_ctypes

        _hooks.set_axon_ntff_profile_hook(
            _ntff_profile_via_ctypes("/opt/axon/libaxon_pjrt.so")
        )
    except Exception:
        pass

import numpy as np
import concourse.bass as bass
import concourse.bacc as bacc
import concourse.mybir as mybir
import concourse.tile as tile
from concourse import masks
from concourse.bass_utils import run_bass_kernel_spmd

F32 = mybir.dt.float32
F32R = mybir.dt.float32r
BF16 = mybir.dt.bfloat16
EXP = mybir.ActivationFunctionType.Exp

B = 8
N = 1024  # sequence length
D = 1024  # embed dim
H = 16  # heads
HD = 64  # head dim
SCALE = HD**-0.5  # 0.125
NT = N // 128  # 8 seq tiles
DT = D // 128  # 8 embed tiles
NC2 = N // 512  # 2 free-dim chunks of 512

LAST_EXEC_NS = [None]


def build():
    nc = bacc.Bacc(None, target_bir_lowering=False)
    x = nc.declare_dram_parameter("x", [N, D], F32, isOutput=False)
    w_qkv = nc.declare_dram_parameter("w_qkv", [D, 3 * D], F32, isOutput=False)
    w_proj = nc.declare_dram_parameter("w_proj", [D, D], F32, isOutput=False)
    b_proj = nc.declare_dram_parameter("b_proj", [D], F32, isOutput=False)
    out = nc.declare_dram_parameter("out", [N, D], F32, isOutput=True)

    with tile.TileContext(nc) as tc:
        # ---- whole-kernel pools --------------------------------------
        with (
            tc.tile_pool(name="const", bufs=1) as cpool,
            tc.tile_pool(name="xT", bufs=DT) as xTpool,
            tc.tile_pool(name="V", bufs=NT) as Vpool,
            tc.tile_pool(name="qkT", bufs=4) as qkTpool,
            tc.tile_pool(name="Onorm", bufs=DT) as Opool,
            tc.tile_pool(name="es", bufs=6) as espool,
        ):
            ident = cpool.tile([128, 128], F32, tag="ident")
            masks.make_identity(nc, ident[:])
            ones1f = cpool.tile([1, 128], F32, tag="ones1f")
            nc.vector.memset(ones1f[:], 1.0)
            ones1 = cpool.tile([1, 128], F32R, tag="ones1")
            nc.vector.tensor_copy(ones1[:], ones1f[:])
            onescf = cpool.tile([128, H], F32, tag="onescf")
            nc.vector.memset(onescf[:], 1.0)
            b_sb = cpool.tile([1, D], F32R, tag="b_sb")
            nc.sync.dma_start(
                b_sb[:], b_proj[:].rearrange("(a n) -> a n", a=1).bitcast(F32R)
            )

            xT = [xTpool.tile([128, N], F32R, tag="xT", name=f"xT{j}") for j in range(DT)]
            V = [Vpool.tile([128, H * (HD + 1)], BF16, tag="V", name=f"V{i}") for i in range(NT)]
            Onorm = [Opool.tile([128, N], F32R, tag="On", name=f"On{i}") for i in range(DT)]

            # ---- phase A: load + transpose x; phase B1: V projection --
            with (
                tc.tile_pool(name="xin", bufs=3) as xpool,
                tc.tile_pool(name="wv", bufs=DT) as wvpool,
                tc.tile_pool(name="tp", bufs=2, space="PSUM") as tppool,
                tc.tile_pool(name="vp", bufs=3, space="PSUM") as vppool,
            ):
                # w_v tiles: w_qkv[j*128:(j+1)*128, 2048:3072] -> [128, 1024]
                wv = []
                for j in range(DT):
                    t = wvpool.tile([128, D], F32R, tag="wv")
                    nc.sync.dma_start(
                        t[:],
                        w_qkv[j * 128 : (j + 1) * 128, 2 * D : 3 * D].bitcast(F32R),
                    )
                    wv.append(t)

                for i in range(NT):
                    xt = xpool.tile([128, D], F32, tag="x")
                    nc.sync.dma_start(xt[:], x[i * 128 : (i + 1) * 128, :])
                    for j in range(DT):
                        tp = tppool.tile([128, 128], F32, tag="tp")
                        nc.tensor.transpose(
                            tp[:], xt[:, j * 128 : (j + 1) * 128], ident[:]
                        )
                        nc.vector.tensor_copy(
                            xT[j][:, i * 128 : (i + 1) * 128], tp[:]
                        )

                # ones columns of V' (col 64 of each 65-wide head group)
                for i in range(NT):
                    ones_view = V[i][:].rearrange("p (h e) -> p h e", e=HD + 1)
                    nc.vector.tensor_copy(
                        ones_view[:, :, HD : HD + 1],
                        onescf[:].rearrange("p (h e) -> p h e", e=1),
                    )

                # V projection: V[n, e] = sum_d x[n, d] w_v[d, e]
                for i in range(NT):
                    pv = [
                        vppool.tile([128, 512], F32, tag="vp", name=f"vp{c}")
                        for c in range(NC2)
                    ]
                    for j in range(DT):
                        for c in range(NC2):
                            nc.tensor.matmul(
                                pv[c][:],
                                xT[j][:, i * 128 : (i + 1) * 128],
                                wv[j][:, c * 512 : (c + 1) * 512],
                                start=(j == 0),
                                stop=(j == DT - 1),
                            )
                    dst = V[i][:].rearrange("p (h e) -> p h e", e=HD + 1)
                    for c in range(NC2):
                        nc.vector.tensor_copy(
                            dst[:, 8 * c : 8 * c + 8, 0:HD],
                            pv[c][:].rearrange("p (h e) -> p h e", e=HD),
                        )

            # ---- phases B2 + C: per head-pair qk projection + attention
            with (
                tc.tile_pool(name="wp", bufs=DT) as wppool,
                tc.tile_pool(name="wqk", bufs=4) as wqkpool,
                tc.tile_pool(name="sums", bufs=1) as sumspool,
                tc.tile_pool(name="srow", bufs=4) as srowpool,
                tc.tile_pool(name="recb", bufs=4) as rbpool,
                tc.tile_pool(name="drs", bufs=1, space="DRAM") as drpool,
            ):
                wp = []
                sums_st = [
                    sumspool.tile([8, 512], F32, tag="sums", name=f"sums{k}", bufs=4)
                    for k in range(4)
                ]
                rec_st = [
                    sumspool.tile([8, 512], F32, tag="recall", name=f"rec{k}", bufs=4)
                    for k in range(4)
                ]
                sums_dram = drpool.tile([32, 512], F32, tag="sumsd")
                rec_dram = drpool.tile([32, 512], F32, tag="recd")

                def recip_stage(stage):
                    # rows for pairs (2*stage, 2*stage+1): 8 rows
                    r0 = 8 * stage
                    nc.sync.dma_start(sums_st[stage][:], sums_dram[r0 : r0 + 8, :])
                    nc.vector.reciprocal(rec_st[stage][:], sums_st[stage][:])
                    nc.sync.dma_start(rec_dram[r0 : r0 + 8, :], rec_st[stage][:])

                def mul_pair(pr):
                    for parity in range(2):
                        h = 2 * pr + parity
                        p0 = 64 * parity
                        for c in range(NC2):
                            hc = h * 2 + c
                            recb = rbpool.tile([128, 512], F32, tag="recb")
                            nc.sync.dma_start(
                                recb[:].rearrange("p (a f) -> p a f", a=1),
                                rec_dram[hc : hc + 1, :].partition_broadcast(128),
                            )
                            osl = Onorm[pr][
                                p0 : p0 + 64, c * 512 : (c + 1) * 512
                            ]
                            nc.vector.tensor_mul(osl, osl, recb[p0 : p0 + 64, :])

                with (
                    tc.tile_pool(name="s", bufs=2, space="PSUM") as spool,
                    tc.tile_pool(name="o", bufs=2, space="PSUM") as opool,
                    tc.tile_pool(name="pq", bufs=2, space="PSUM") as pqpool,
                ):
                    def make_qk_emitter(pr):
                        # prepares qkproj work for pair `pr`; emit(n) issues n
                        # matmuls (32 total = 2 e-tiles x 2 chunks x 8 j)
                        wqs = []
                        for et in (pr, DT + pr):
                            wq = wqkpool.tile(
                                [128, D], F32R, tag="wqk", name=f"wq{et}"
                            )
                            src_w = w_qkv[:, et * 128 : (et + 1) * 128].rearrange(
                                "(j p) e -> p j e", p=128
                            )
                            nc.sync.dma_start(
                                wq[:].rearrange("p (j e) -> p j e", e=128),
                                src_w.bitcast(F32R),
                            )
                            wqs.append(wq)
                        qts = [
                            qkTpool.tile([128, N], BF16, tag="qkT", name=f"qt{pr}"),
                            qkTpool.tile([128, N], BF16, tag="qkT", name=f"kt{pr}"),
                        ]
                        state = {"idx": 0, "pq": None}

                        def emit(n):
                            for _ in range(n):
                                idx = state["idx"]
                                if idx >= 32:
                                    return
                                g, j = idx // DT, idx % DT
                                ei, c = g // 2, g % 2
                                if j == 0:
                                    state["pq"] = pqpool.tile(
                                        [128, 512], F32, tag="pq", name="pq"
                                    )
                                nc.tensor.matmul(
                                    state["pq"][:],
                                    wqs[ei][:, j * 128 : (j + 1) * 128],
                                    xT[j][:, c * 512 : (c + 1) * 512],
                                    start=(j == 0),
                                    stop=(j == DT - 1),
                                )
                                if j == DT - 1:
                                    nc.vector.tensor_copy(
                                        qts[ei][:, c * 512 : (c + 1) * 512],
                                        state["pq"][:],
                                    )
                                state["idx"] = idx + 1

                        return emit, qts

                    emit0, qts0 = make_qk_emitter(0)
                    emit0(32)
                    cur_qts = qts0
                    for pair in range(H // 2):
                        if pair == 2:
                            for ct in range(DT):
                                t = wppool.tile(
                                    [128, D], F32R, tag="wp", name=f"wp{ct}"
                                )
                                nc.sync.dma_start(
                                    t[:],
                                    w_proj[ct * 128 : (ct + 1) * 128, :].bitcast(
                                        F32R
                                    ),
                                )
                                wp.append(t)
                        if pair + 1 < H // 2:
                            emit_next, next_qts = make_qk_emitter(pair + 1)
                        else:
                            emit_next, next_qts = (lambda n: None), None
                        qt_pair, kt_pair = cur_qts

                        # attention for both heads of the pair, chunk passes
                        for cpass in range(NC2):
                            cs = slice(cpass * 512, (cpass + 1) * 512)
                            po = [
                                opool.tile([128, 512], F32, tag="o", name=f"po{p}")
                                for p in range(2)
                            ]
                            for j in range(NT):
                                st = spool.tile([128, N], F32, tag="s", name="st")
                                es = espool.tile([128, N], BF16, tag="es")
                                for parity in range(2):
                                    p0 = 64 * parity
                                    kT = kt_pair[p0 : p0 + 64, :]
                                    qT = qt_pair[p0 : p0 + 64, :]
                                    nc.tensor.matmul(
                                        st[:, 512 * parity : 512 * parity + 512],
                                        kT[:, j * 128 : (j + 1) * 128],
                                        qT[:, cs],
                                        start=True,
                                        stop=True,
                                    )
                                nc.scalar.activation(es[:], st[:], EXP, scale=SCALE)
                                for parity in range(2):
                                    h = 2 * pair + parity
                                    nc.tensor.matmul(
                                        po[parity][0 : HD + 1, :],
                                        V[j][:, h * (HD + 1) : (h + 1) * (HD + 1)],
                                        es[:, 512 * parity : 512 * parity + 512],
                                        start=(j == 0),
                                        stop=(j == NT - 1),
                                    )
                                emit_next(2)
                            for parity in range(2):
                                h = 2 * pair + parity
                                p0 = 64 * parity
                                osl = Onorm[pair][p0 : p0 + 64, cs]
                                nc.vector.tensor_copy(osl, po[parity][0:HD, :])
                                hc = h * 2 + cpass
                                srow = srowpool.tile([1, 512], F32, tag="srow")
                                nc.vector.tensor_copy(
                                    srow[:], po[parity][HD : HD + 1, :]
                                )
                                nc.sync.dma_start(
                                    sums_dram[hc : hc + 1, :], srow[:]
                                )
                        cur_qts = next_qts
                        if pair % 2 == 1:
                            recip_stage(pair // 2)
                        if pair >= 2:
                            mul_pair(pair - 2)
                    for pr in (6, 7):
                        mul_pair(pr)

                # ---- phase D: output projection + bias ------------------
                # ct-outer so only the last 8 matmuls wait on the final
                # pair's normalization; i processed in halves of 4 (8 banks)
                with (
                    tc.tile_pool(name="osb", bufs=3) as osbpool,
                    tc.tile_pool(name="dp", bufs=8, space="PSUM") as dppool,
                ):
                    for half in range(2):
                        i0 = 4 * half
                        pd = [
                            dppool.tile([128, 512], F32, tag="dp", name=f"dp{k}")
                            for k in range(8)
                        ]
                        for ct in range(DT):
                            for di in range(4):
                                i = i0 + di
                                for ec in range(NC2):
                                    nc.tensor.matmul(
                                        pd[2 * di + ec][:],
                                        Onorm[ct][:, i * 128 : (i + 1) * 128],
                                        wp[ct][:, ec * 512 : (ec + 1) * 512],
                                        start=(ct == 0),
                                        stop=False,
                                    )
                        for di in range(4):
                            i = i0 + di
                            ot = osbpool.tile([128, D], F32, tag="osb")
                            for ec in range(NC2):
                                nc.tensor.matmul(
                                    pd[2 * di + ec][:],
                                    ones1[:],
                                    b_sb[:, ec * 512 : (ec + 1) * 512],
                                    start=False,
                                    stop=True,
                                )
                                nc.vector.tensor_copy(
                                    ot[:, ec * 512 : (ec + 1) * 512],
                                    pd[2 * di + ec][:],
                                )
                            nc.sync.dma_start(
                                out[i * 128 : (i + 1) * 128, :], ot[:]
                            )

    nc.compile()
    return nc


_NC = [None]


def _get_nc():
    if _NC[0] is None:
        _NC[0] = build()
    return _NC[0]


def kernel(x, w_qkv, w_proj, b_proj):
    x = np.asarray(x, dtype=np.float32)
    w_qkv = np.asarray(w_qkv, dtype=np.float32)
    w_proj = np.asarray(w_proj, dtype=np.float32)
    b_proj = np.asarray(b_proj, dtype=np.float32)
    assert x.shape == (B, N, D)

    nc = _get_nc()
    in_maps = [
        {"x": x[i], "w_qkv": w_qkv, "w_proj": w_proj, "b_proj": b_proj}
        for i in range(B)
    ]
    trace = os.environ.get("KERNEL_TRACE") == "1"
    res = run_bass_kernel_spmd(
        nc, in_maps, core_ids=list(range(B)), trace=trace
    )
    LAST_EXEC_NS[0] = res.exec_time_ns
    return np.stack([res.results[i]["out"] for i in range(B)], axis=0)


# revision 24
# speedup vs baseline: 1.8874x; 1.0210x over previous
"""Multi-head attention (B=8, N=1024, D=1024, H=16) on 8 TRN2 NeuronCores.

Sharding: data-parallel over batch — core i computes batch item i end-to-end.
No collectives. Per-core pipeline (all matmuls in float32r = full PE rate):

  A)  transpose x [N,D] -> xT [D,N]            (PE transpose via identity)
  B1) V = x @ w_v                 (lhsT=xT tiles, rhs=w_v) -> V natural [k,d]
      stored with a ones column per head: V'[k, 65] = [V_h | 1]
  B2) qkT = w_qk^T @ x^T          (lhsT=w_qk tiles, rhs=xT) -> qT,kT [64,N]/head
  C)  per head h: S^T[k,q] = kT^T qT  (K=64, row-packed pairs via base partition)
      expS = exp(0.125*S^T)  (ACT, PSUM->SBUF, f32r out)
      O'^T[65, q] = sum_k V'_h^T expS  -> rows 0..63 = O^T, row 64 = softmax sums
      normalize: O^T * (1/sums)  broadcast along partitions
  D)  out = Onorm^T^T @ w_proj + b  (lhsT=Onorm tiles, rhs=w_proj; bias via
      rank-1 ones x b matmul into the same PSUM accumulation)

softmax max-subtraction is skipped: scores are ~N(0, 0.33^2) here, bounded
well inside exp's fp32 range, so exp/sum is exact enough (matches reference
mathematically).
"""

import os
import sys
import types

sys.path.insert(0, "/opt/trn_rl_repo")

# The agent image's antenv lacks axon_hooks; register the NTFF profile hook
# shim so run_bass_kernel_spmd(trace=True) can capture exec_time_ns.
if "antenv.axon_hooks" not in sys.modules:
    _hooks = types.ModuleType("antenv.axon_hooks")
    _hook_store = [None]
    _hooks.set_axon_ntff_profile_hook = lambda h: _hook_store.__setitem__(0, h)
    _hooks.get_axon_ntff_profile_hook = lambda: _hook_store[0]
    sys.modules["antenv.axon_hooks"] = _hooks
    try:
        from trn_agent_boot.trn_boot import _ntff_profile_via_ctypes

        _hooks.set_axon_ntff_profile_hook(
            _ntff_profile_via_ctypes("/opt/axon/libaxon_pjrt.so")
        )
    except Exception:
        pass

import numpy as np
import concourse.bass as bass
import concourse.bacc as bacc
import concourse.mybir as mybir
import concourse.tile as tile
from concourse import masks
from concourse.bass_utils import run_bass_kernel_spmd

F32 = mybir.dt.float32
F32R = mybir.dt.float32r
BF16 = mybir.dt.bfloat16
EXP = mybir.ActivationFunctionType.Exp

B = 8
N = 1024  # sequence length
D = 1024  # embed dim
H = 16  # heads
HD = 64  # head dim
SCALE = HD**-0.5  # 0.125
NT = N // 128  # 8 seq tiles
DT = D // 128  # 8 embed tiles
NC2 = N // 512  # 2 free-dim chunks of 512

LAST_EXEC_NS = [None]


def build():
    nc = bacc.Bacc(None, target_bir_lowering=False)
    x = nc.declare_dram_parameter("x", [N, D], F32, isOutput=False)
    w_qkv = nc.declare_dram_parameter("w_qkv", [D, 3 * D], F32, isOutput=False)
    w_proj = nc.declare_dram_parameter("w_proj", [D, D], F32, isOutput=False)
    b_proj = nc.declare_dram_parameter("b_proj", [D], F32, isOutput=False)
    out = nc.declare_dram_parameter("out", [N, D], F32, isOutput=True)

    with tile.TileContext(nc) as tc:
        # ---- whole-kernel pools --------------------------------------
        with (
            tc.tile_pool(name="const", bufs=1) as cpool,
            tc.tile_pool(name="xT", bufs=DT) as xTpool,
            tc.tile_pool(name="V", bufs=NT) as Vpool,
            tc.tile_pool(name="qkT", bufs=4) as qkTpool,
            tc.tile_pool(name="Onorm", bufs=DT) as Opool,
            tc.tile_pool(name="es", bufs=4) as espool,
        ):
            ident = cpool.tile([128, 128], F32, tag="ident")
            masks.make_identity(nc, ident[:])
            ones1f = cpool.tile([1, 128], F32, tag="ones1f")
            nc.vector.memset(ones1f[:], 1.0)
            ones1 = cpool.tile([1, 128], F32R, tag="ones1")
            nc.vector.tensor_copy(ones1[:], ones1f[:])
            onescf = cpool.tile([128, H], F32, tag="onescf")
            nc.vector.memset(onescf[:], 1.0)
            b_sb = cpool.tile([1, D], F32R, tag="b_sb")
            nc.sync.dma_start(
                b_sb[:], b_proj[:].rearrange("(a n) -> a n", a=1).bitcast(F32R)
            )

            xT = [xTpool.tile([128, N], F32R, tag="xT", name=f"xT{j}") for j in range(DT)]
            V = [Vpool.tile([128, H * (HD + 1)], BF16, tag="V", name=f"V{i}") for i in range(NT)]
            Onorm = [Opool.tile([128, N], F32R, tag="On", name=f"On{i}") for i in range(DT)]

            # ---- phase A: load + transpose x; phase B1: V projection --
            with (
                tc.tile_pool(name="xin", bufs=3) as xpool,
                tc.tile_pool(name="wv", bufs=DT) as wvpool,
                tc.tile_pool(name="tp", bufs=2, space="PSUM") as tppool,
                tc.tile_pool(name="vp", bufs=3, space="PSUM") as vppool,
            ):
                # w_v tiles: w_qkv[j*128:(j+1)*128, 2048:3072] -> [128, 1024]
                wv = []
                for j in range(DT):
                    t = wvpool.tile([128, D], F32R, tag="wv")
                    nc.sync.dma_start(
                        t[:],
                        w_qkv[j * 128 : (j + 1) * 128, 2 * D : 3 * D].bitcast(F32R),
                    )
                    wv.append(t)

                for i in range(NT):
                    xt = xpool.tile([128, D], F32, tag="x")
                    nc.sync.dma_start(xt[:], x[i * 128 : (i + 1) * 128, :])
                    for j in range(DT):
                        tp = tppool.tile([128, 128], F32, tag="tp")
                        nc.tensor.transpose(
                            tp[:], xt[:, j * 128 : (j + 1) * 128], ident[:]
                        )
                        nc.vector.tensor_copy(
                            xT[j][:, i * 128 : (i + 1) * 128], tp[:]
                        )

                # ones columns of V' (col 64 of each 65-wide head group)
                for i in range(NT):
                    ones_view = V[i][:].rearrange("p (h e) -> p h e", e=HD + 1)
                    nc.vector.tensor_copy(
                        ones_view[:, :, HD : HD + 1],
                        onescf[:].rearrange("p (h e) -> p h e", e=1),
                    )

                # V projection: V[n, e] = sum_d x[n, d] w_v[d, e]
                for i in range(NT):
                    pv = [
                        vppool.tile([128, 512], F32, tag="vp", name=f"vp{c}")
                        for c in range(NC2)
                    ]
                    for j in range(DT):
                        for c in range(NC2):
                            nc.tensor.matmul(
                                pv[c][:],
                                xT[j][:, i * 128 : (i + 1) * 128],
                                wv[j][:, c * 512 : (c + 1) * 512],
                                start=(j == 0),
                                stop=(j == DT - 1),
                            )
                    dst = V[i][:].rearrange("p (h e) -> p h e", e=HD + 1)
                    for c in range(NC2):
                        nc.vector.tensor_copy(
                            dst[:, 8 * c : 8 * c + 8, 0:HD],
                            pv[c][:].rearrange("p (h e) -> p h e", e=HD),
                        )

            # ---- phases B2 + C: per head-pair qk projection + attention
            with (
                tc.tile_pool(name="wp", bufs=DT) as wppool,
                tc.tile_pool(name="wqk", bufs=4) as wqkpool,
                tc.tile_pool(name="sums", bufs=1) as sumspool,
                tc.tile_pool(name="srow", bufs=4) as srowpool,
                tc.tile_pool(name="recb", bufs=4) as rbpool,
                tc.tile_pool(name="drs", bufs=1, space="DRAM") as drpool,
            ):
                wp = []
                sums_st = [
                    sumspool.tile([8, 512], F32, tag="sums", name=f"sums{k}", bufs=4)
                    for k in range(4)
                ]
                rec_st = [
                    sumspool.tile([8, 512], F32, tag="recall", name=f"rec{k}", bufs=4)
                    for k in range(4)
                ]
                sums_dram = drpool.tile([32, 512], F32, tag="sumsd")
                rec_dram = drpool.tile([32, 512], F32, tag="recd")

                def recip_stage(stage):
                    # rows for pairs (2*stage, 2*stage+1): 8 rows
                    r0 = 8 * stage
                    nc.sync.dma_start(sums_st[stage][:], sums_dram[r0 : r0 + 8, :])
                    nc.vector.reciprocal(rec_st[stage][:], sums_st[stage][:])
                    nc.sync.dma_start(rec_dram[r0 : r0 + 8, :], rec_st[stage][:])

                def mul_pair(pr):
                    for parity in range(2):
                        h = 2 * pr + parity
                        p0 = 64 * parity
                        for c in range(NC2):
                            hc = h * 2 + c
                            recb = rbpool.tile([128, 512], F32, tag="recb")
                            nc.sync.dma_start(
                                recb[:].rearrange("p (a f) -> p a f", a=1),
                                rec_dram[hc : hc + 1, :].partition_broadcast(128),
                            )
                            osl = Onorm[pr][
                                p0 : p0 + 64, c * 512 : (c + 1) * 512
                            ]
                            nc.vector.tensor_mul(osl, osl, recb[p0 : p0 + 64, :])

                with (
                    tc.tile_pool(name="s", bufs=2, space="PSUM") as spool,
                    tc.tile_pool(name="o", bufs=2, space="PSUM") as opool,
                    tc.tile_pool(name="pq", bufs=2, space="PSUM") as pqpool,
                ):
                    def make_qk_emitter(pr):
                        # prepares qkproj work for pair `pr`; emit(n) issues n
                        # matmuls (32 total = 2 e-tiles x 2 chunks x 8 j)
                        wqs = []
                        for et in (pr, DT + pr):
                            wq = wqkpool.tile(
                                [128, D], F32R, tag="wqk", name=f"wq{et}"
                            )
                            src_w = w_qkv[:, et * 128 : (et + 1) * 128].rearrange(
                                "(j p) e -> p j e", p=128
                            )
                            nc.sync.dma_start(
                                wq[:].rearrange("p (j e) -> p j e", e=128),
                                src_w.bitcast(F32R),
                            )
                            wqs.append(wq)
                        qts = [
                            qkTpool.tile([128, N], BF16, tag="qkT", name=f"qt{pr}"),
                            qkTpool.tile([128, N], BF16, tag="qkT", name=f"kt{pr}"),
                        ]
                        state = {"idx": 0, "pq": None}

                        def emit(n):
                            for _ in range(n):
                                idx = state["idx"]
                                if idx >= 32:
                                    return
                                g, j = idx // DT, idx % DT
                                ei, c = g // 2, g % 2
                                if j == 0:
                                    state["pq"] = pqpool.tile(
                                        [128, 512], F32, tag="pq", name="pq"
                                    )
                                nc.tensor.matmul(
                                    state["pq"][:],
                                    wqs[ei][:, j * 128 : (j + 1) * 128],
                                    xT[j][:, c * 512 : (c + 1) * 512],
                                    start=(j == 0),
                                    stop=(j == DT - 1),
                                )
                                if j == DT - 1:
                                    nc.vector.tensor_copy(
                                        qts[ei][:, c * 512 : (c + 1) * 512],
                                        state["pq"][:],
                                    )
                                state["idx"] = idx + 1

                        return emit, qts

                    emit0, qts0 = make_qk_emitter(0)
                    emit0(32)
                    cur_qts = qts0
                    for pair in range(H // 2):
                        if pair == 2:
                            for ct in range(DT):
                                t = wppool.tile(
                                    [128, D], F32R, tag="wp", name=f"wp{ct}"
                                )
                                nc.sync.dma_start(
                                    t[:],
                                    w_proj[ct * 128 : (ct + 1) * 128, :].bitcast(
                                        F32R
                                    ),
                                )
                                wp.append(t)
                        if pair + 1 < H // 2:
                            emit_next, next_qts = make_qk_emitter(pair + 1)
                        else:
                            emit_next, next_qts = (lambda n: None), None
                        qt_pair, kt_pair = cur_qts

                        # attention for both heads of the pair, chunk passes
                        for cpass in range(NC2):
                            cs = slice(cpass * 512, (cpass + 1) * 512)
                            po = [
                                opool.tile([128, 512], F32, tag="o", name=f"po{p}")
                                for p in range(2)
                            ]
                            for j in range(NT):
                                st = spool.tile([128, N], F32, tag="s", name="st")
                                es = espool.tile([128, N], BF16, tag="es")
                                for parity in range(2):
                                    p0 = 64 * parity
                                    kT = kt_pair[p0 : p0 + 64, :]
                                    qT = qt_pair[p0 : p0 + 64, :]
                                    nc.tensor.matmul(
                                        st[:, 512 * parity : 512 * parity + 512],
                                        kT[:, j * 128 : (j + 1) * 128],
                                        qT[:, cs],
                                        start=True,
                                        stop=True,
                                    )
                                nc.scalar.activation(es[:], st[:], EXP, scale=SCALE)
                                for parity in range(2):
                                    h = 2 * pair + parity
                                    nc.tensor.matmul(
                                        po[parity][0 : HD + 1, :],
                                        V[j][:, h * (HD + 1) : (h + 1) * (HD + 1)],
                                        es[:, 512 * parity : 512 * parity + 512],
                                        start=(j == 0),
                                        stop=(j == NT - 1),
                                    )
                                emit_next(2)
                            for parity in range(2):
                                h = 2 * pair + parity
                                p0 = 64 * parity
                                osl = Onorm[pair][p0 : p0 + 64, cs]
                                nc.vector.tensor_copy(osl, po[parity][0:HD, :])
                                hc = h * 2 + cpass
                                srow = srowpool.tile([1, 512], F32, tag="srow")
                                nc.vector.tensor_copy(
                                    srow[:], po[parity][HD : HD + 1, :]
                                )
                                nc.sync.dma_start(
                                    sums_dram[hc : hc + 1, :], srow[:]
                                )
                        cur_qts = next_qts
                        if pair % 2 == 1:
                            recip_stage(pair // 2)
                        if pair >= 2:
                            mul_pair(pair - 2)
                    for pr in (6, 7):
                        mul_pair(pr)

                # ---- phase D: output projection + bias ------------------
                # ct-outer so only the last 8 matmuls wait on the final
                # pair's normalization; i processed in halves of 4 (8 banks)
                with (
                    tc.tile_pool(name="osb", bufs=2) as osbpool,
                    tc.tile_pool(name="dp", bufs=8, space="PSUM") as dppool,
                ):
                    for half in range(2):
                        i0 = 4 * half
                        pd = [
                            dppool.tile([128, 512], F32, tag="dp", name=f"dp{k}")
                            for k in range(8)
                        ]
                        for ct in range(DT):
                            for di in range(4):
                                i = i0 + di
                                for ec in range(NC2):
                                    nc.tensor.matmul(
                                        pd[2 * di + ec][:],
                                        Onorm[ct][:, i * 128 : (i + 1) * 128],
                                        wp[ct][:, ec * 512 : (ec + 1) * 512],
                                        start=(ct == 0),
                                        stop=False,
                                    )
                        for di in range(4):
                            i = i0 + di
                            ot = osbpool.tile([128, D], F32, tag="osb")
                            for ec in range(NC2):
                                nc.tensor.matmul(
                                    pd[2 * di + ec][:],
                                    ones1[:],
                                    b_sb[:, ec * 512 : (ec + 1) * 512],
                                    start=False,
                                    stop=True,
                                )
                                nc.vector.tensor_copy(
                                    ot[:, ec * 512 : (ec + 1) * 512],
                                    pd[2 * di + ec][:],
                                )
                            nc.sync.dma_start(
                                out[i * 128 : (i + 1) * 128, :], ot[:]
                            )

    nc.compile()
    return nc


_NC = [None]


def _get_nc():
    if _NC[0] is None:
        _NC[0] = build()
    return _NC[0]


def kernel(x, w_qkv, w_proj, b_proj):
    x = np.asarray(x, dtype=np.float32)
    w_qkv = np.asarray(w_qkv, dtype=np.float32)
    w_proj = np.asarray(w_proj, dtype=np.float32)
    b_proj = np.asarray(b_proj, dtype=np.float32)
    assert x.shape == (B, N, D)

    nc = _get_nc()
    in_maps = [
        {"x": x[i], "w_qkv": w_qkv, "w_proj": w_proj, "b_proj": b_proj}
        for i in range(B)
    ]
    trace = os.environ.get("KERNEL_TRACE") == "1"
    res = run_bass_kernel_spmd(
        nc, in_maps, core_ids=list(range(B)), trace=trace
    )
    LAST_EXEC_NS[0] = res.exec_time_ns
    return np.stack([res.results[i]["out"] for i in range(B)], axis=0)


# revision 28
# speedup vs baseline: 1.9026x; 1.0080x over previous
"""Multi-head attention (B=8, N=1024, D=1024, H=16) on 8 TRN2 NeuronCores.

Sharding: data-parallel over batch — core i computes batch item i end-to-end.
No collectives. Per-core pipeline (all matmuls in float32r = full PE rate):

  A)  transpose x [N,D] -> xT [D,N]            (PE transpose via identity)
  B1) V = x @ w_v                 (lhsT=xT tiles, rhs=w_v) -> V natural [k,d]
      stored with a ones column per head: V'[k, 65] = [V_h | 1]
  B2) qkT = w_qk^T @ x^T          (lhsT=w_qk tiles, rhs=xT) -> qT,kT [64,N]/head
  C)  per head h: S^T[k,q] = kT^T qT  (K=64, row-packed pairs via base partition)
      expS = exp(0.125*S^T)  (ACT, PSUM->SBUF, f32r out)
      O'^T[65, q] = sum_k V'_h^T expS  -> rows 0..63 = O^T, row 64 = softmax sums
      normalize: O^T * (1/sums)  broadcast along partitions
  D)  out = Onorm^T^T @ w_proj + b  (lhsT=Onorm tiles, rhs=w_proj; bias via
      rank-1 ones x b matmul into the same PSUM accumulation)

softmax max-subtraction is skipped: scores are ~N(0, 0.33^2) here, bounded
well inside exp's fp32 range, so exp/sum is exact enough (matches reference
mathematically).
"""

import os
import sys
import types

sys.path.insert(0, "/opt/trn_rl_repo")

# The agent image's antenv lacks axon_hooks; register the NTFF profile hook
# shim so run_bass_kernel_spmd(trace=True) can capture exec_time_ns.
if "antenv.axon_hooks" not in sys.modules:
    _hooks = types.ModuleType("antenv.axon_hooks")
    _hook_store = [None]
    _hooks.set_axon_ntff_profile_hook = lambda h: _hook_store.__setitem__(0, h)
    _hooks.get_axon_ntff_profile_hook = lambda: _hook_store[0]
    sys.modules["antenv.axon_hooks"] = _hooks
    try:
        from trn_agent_boot.trn_boot import _ntff_profile_via_ctypes

        _hooks.set_axon_ntff_profile_hook(
            _ntff_profile_via_ctypes("/opt/axon/libaxon_pjrt.so")
        )
    except Exception:
        pass

import numpy as np
import concourse.bass as bass
import concourse.bacc as bacc
import concourse.mybir as mybir
import concourse.tile as tile
from concourse import masks
from concourse.bass_utils import run_bass_kernel_spmd

F32 = mybir.dt.float32
F32R = mybir.dt.float32r
BF16 = mybir.dt.bfloat16
EXP = mybir.ActivationFunctionType.Exp

B = 8
N = 1024  # sequence length
D = 1024  # embed dim
H = 16  # heads
HD = 64  # head dim
SCALE = HD**-0.5  # 0.125
NT = N // 128  # 8 seq tiles
DT = D // 128  # 8 embed tiles
NC2 = N // 512  # 2 free-dim chunks of 512

LAST_EXEC_NS = [None]


def build():
    nc = bacc.Bacc(None, target_bir_lowering=False)
    x = nc.declare_dram_parameter("x", [N, D], F32, isOutput=False)
    w_qkv = nc.declare_dram_parameter("w_qkv", [D, 3 * D], F32, isOutput=False)
    w_proj = nc.declare_dram_parameter("w_proj", [D, D], F32, isOutput=False)
    b_proj = nc.declare_dram_parameter("b_proj", [D], F32, isOutput=False)
    out = nc.declare_dram_parameter("out", [N, D], F32, isOutput=True)

    with tile.TileContext(nc) as tc:
        # ---- whole-kernel pools --------------------------------------
        with (
            tc.tile_pool(name="const", bufs=1) as cpool,
            tc.tile_pool(name="xT", bufs=DT) as xTpool,
            tc.tile_pool(name="V", bufs=NT) as Vpool,
            tc.tile_pool(name="qkT", bufs=4) as qkTpool,
            tc.tile_pool(name="Onorm", bufs=DT) as Opool,
            tc.tile_pool(name="es", bufs=4) as espool,
        ):
            ident = cpool.tile([128, 128], F32, tag="ident")
            masks.make_identity(nc, ident[:])
            ones1f = cpool.tile([1, 128], F32, tag="ones1f")
            nc.vector.memset(ones1f[:], 1.0)
            ones1 = cpool.tile([1, 128], F32R, tag="ones1")
            nc.vector.tensor_copy(ones1[:], ones1f[:])
            onescf = cpool.tile([128, H], F32, tag="onescf")
            nc.vector.memset(onescf[:], 1.0)
            b_sb = cpool.tile([1, D], F32R, tag="b_sb")
            nc.sync.dma_start(
                b_sb[:], b_proj[:].rearrange("(a n) -> a n", a=1).bitcast(F32R)
            )

            xT = [xTpool.tile([128, N], F32R, tag="xT", name=f"xT{j}") for j in range(DT)]
            V = [Vpool.tile([128, H * (HD + 1)], BF16, tag="V", name=f"V{i}") for i in range(NT)]
            Onorm = [Opool.tile([128, N], F32R, tag="On", name=f"On{i}") for i in range(DT)]

            # ---- phase A: load + transpose x; phase B1: V projection --
            with (
                tc.tile_pool(name="xin", bufs=3) as xpool,
                tc.tile_pool(name="wv", bufs=DT) as wvpool,
                tc.tile_pool(name="tp", bufs=2, space="PSUM") as tppool,
                tc.tile_pool(name="vp", bufs=3, space="PSUM") as vppool,
            ):
                # w_v tiles: w_qkv[j*128:(j+1)*128, 2048:3072] -> [128, 1024]
                wv = []
                for j in range(DT):
                    t = wvpool.tile([128, D], F32R, tag="wv")
                    nc.sync.dma_start(
                        t[:],
                        w_qkv[j * 128 : (j + 1) * 128, 2 * D : 3 * D].bitcast(F32R),
                    )
                    wv.append(t)

                for i in range(NT):
                    xt = xpool.tile([128, D], F32, tag="x")
                    nc.sync.dma_start(xt[:], x[i * 128 : (i + 1) * 128, :])
                    for j in range(DT):
                        tp = tppool.tile([128, 128], F32, tag="tp")
                        nc.tensor.transpose(
                            tp[:], xt[:, j * 128 : (j + 1) * 128], ident[:]
                        )
                        nc.vector.tensor_copy(
                            xT[j][:, i * 128 : (i + 1) * 128], tp[:]
                        )

                # ones columns of V' (col 64 of each 65-wide head group)
                for i in range(NT):
                    ones_view = V[i][:].rearrange("p (h e) -> p h e", e=HD + 1)
                    nc.vector.tensor_copy(
                        ones_view[:, :, HD : HD + 1],
                        onescf[:].rearrange("p (h e) -> p h e", e=1),
                    )

                emit0, qts0 = make_qk_emitter(0)
                # V projection: V[n, e] = sum_d x[n, d] w_v[d, e]
                #   pair 0's qk-projection matmuls interleaved (4 per i)
                for i in range(NT):
                    pv = [
                        vppool.tile([128, 512], F32, tag="vp", name=f"vp{c}")
                        for c in range(NC2)
                    ]
                    for j in range(DT):
                        for c in range(NC2):
                            nc.tensor.matmul(
                                pv[c][:],
                                xT[j][:, i * 128 : (i + 1) * 128],
                                wv[j][:, c * 512 : (c + 1) * 512],
                                start=(j == 0),
                                stop=(j == DT - 1),
                            )
                    dst = V[i][:].rearrange("p (h e) -> p h e", e=HD + 1)
                    for c in range(NC2):
                        nc.vector.tensor_copy(
                            dst[:, 8 * c : 8 * c + 8, 0:HD],
                            pv[c][:].rearrange("p (h e) -> p h e", e=HD),
                        )

            # ---- phases B2 + C: per head-pair qk projection + attention
            with (
                tc.tile_pool(name="wp", bufs=DT) as wppool,
                tc.tile_pool(name="wqk", bufs=4) as wqkpool,
                tc.tile_pool(name="sums", bufs=1) as sumspool,
                tc.tile_pool(name="srow", bufs=4) as srowpool,
                tc.tile_pool(name="recb", bufs=4) as rbpool,
                tc.tile_pool(name="drs", bufs=1, space="DRAM") as drpool,
            ):
                wp = []
                sums_st = [
                    sumspool.tile([8, 512], F32, tag="sums", name=f"sums{k}", bufs=4)
                    for k in range(4)
                ]
                rec_st = [
                    sumspool.tile([8, 512], F32, tag="recall", name=f"rec{k}", bufs=4)
                    for k in range(4)
                ]
                sums_dram = drpool.tile([32, 512], F32, tag="sumsd")
                rec_dram = drpool.tile([32, 512], F32, tag="recd")

                def recip_stage(stage):
                    # rows for pairs (2*stage, 2*stage+1): 8 rows
                    r0 = 8 * stage
                    nc.sync.dma_start(sums_st[stage][:], sums_dram[r0 : r0 + 8, :])
                    nc.vector.reciprocal(rec_st[stage][:], sums_st[stage][:])
                    nc.sync.dma_start(rec_dram[r0 : r0 + 8, :], rec_st[stage][:])

                def mul_pair(pr):
                    for parity in range(2):
                        h = 2 * pr + parity
                        p0 = 64 * parity
                        for c in range(NC2):
                            hc = h * 2 + c
                            recb = rbpool.tile([128, 512], F32, tag="recb")
                            nc.sync.dma_start(
                                recb[:].rearrange("p (a f) -> p a f", a=1),
                                rec_dram[hc : hc + 1, :].partition_broadcast(128),
                            )
                            osl = Onorm[pr][
                                p0 : p0 + 64, c * 512 : (c + 1) * 512
                            ]
                            nc.vector.tensor_mul(osl, osl, recb[p0 : p0 + 64, :])

                with (
                    tc.tile_pool(name="s", bufs=2, space="PSUM") as spool,
                    tc.tile_pool(name="o", bufs=2, space="PSUM") as opool,
                    tc.tile_pool(name="pq", bufs=2, space="PSUM") as pqpool,
                ):
                    def make_qk_emitter(pr):
                        # prepares qkproj work for pair `pr`; emit(n) issues n
                        # matmuls (32 total = 2 e-tiles x 2 chunks x 8 j)
                        wqs = []
                        for et in (pr, DT + pr):
                            wq = wqkpool.tile(
                                [128, D], F32R, tag="wqk", name=f"wq{et}"
                            )
                            src_w = w_qkv[:, et * 128 : (et + 1) * 128].rearrange(
                                "(j p) e -> p j e", p=128
                            )
                            nc.sync.dma_start(
                                wq[:].rearrange("p (j e) -> p j e", e=128),
                                src_w.bitcast(F32R),
                            )
                            wqs.append(wq)
                        qts = [
                            qkTpool.tile([128, N], BF16, tag="qkT", name=f"qt{pr}"),
                            qkTpool.tile([128, N], BF16, tag="qkT", name=f"kt{pr}"),
                        ]
                        state = {"idx": 0, "pq": None}

                        def emit(n):
                            for _ in range(n):
                                idx = state["idx"]
                                if idx >= 32:
                                    return
                                g, j = idx // DT, idx % DT
                                ei, c = g // 2, g % 2
                                if j == 0:
                                    state["pq"] = pqpool.tile(
                                        [128, 512], F32, tag="pq", name="pq"
                                    )
                                nc.tensor.matmul(
                                    state["pq"][:],
                                    wqs[ei][:, j * 128 : (j + 1) * 128],
                                    xT[j][:, c * 512 : (c + 1) * 512],
                                    start=(j == 0),
                                    stop=(j == DT - 1),
                                )
                                if j == DT - 1:
                                    nc.vector.tensor_copy(
                                        qts[ei][:, c * 512 : (c + 1) * 512],
                                        state["pq"][:],
                                    )
                                state["idx"] = idx + 1

                        return emit, qts

                    emit0, qts0 = make_qk_emitter(0)
                    emit0(32)
                    cur_qts = qts0
                    for pair in range(H // 2):
                        if pair == 2:
                            for ct in range(DT):
                                t = wppool.tile(
                                    [128, D], F32R, tag="wp", name=f"wp{ct}"
                                )
                                nc.sync.dma_start(
                                    t[:],
                                    w_proj[ct * 128 : (ct + 1) * 128, :].bitcast(
                                        F32R
                                    ),
                                )
                                wp.append(t)
                        if pair + 1 < H // 2:
                            emit_next, next_qts = make_qk_emitter(pair + 1)
                        else:
                            emit_next, next_qts = (lambda n: None), None
                        qt_pair, kt_pair = cur_qts

                        # attention for both heads of the pair, chunk passes
                        for cpass in range(NC2):
                            cs = slice(cpass * 512, (cpass + 1) * 512)
                            po = [
                                opool.tile([128, 512], F32, tag="o", name=f"po{p}")
                                for p in range(2)
                            ]
                            for j in range(NT):
                                st = spool.tile([128, N], F32, tag="s", name="st")
                                es = espool.tile([128, N], BF16, tag="es")
                                for parity in range(2):
                                    p0 = 64 * parity
                                    kT = kt_pair[p0 : p0 + 64, :]
                                    qT = qt_pair[p0 : p0 + 64, :]
                                    nc.tensor.matmul(
                                        st[:, 512 * parity : 512 * parity + 512],
                                        kT[:, j * 128 : (j + 1) * 128],
                                        qT[:, cs],
                                        start=True,
                                        stop=True,
                                    )
                                nc.scalar.activation(es[:], st[:], EXP, scale=SCALE)
                                for parity in range(2):
                                    h = 2 * pair + parity
                                    nc.tensor.matmul(
                                        po[parity][0 : HD + 1, :],
                                        V[j][:, h * (HD + 1) : (h + 1) * (HD + 1)],
                                        es[:, 512 * parity : 512 * parity + 512],
                                        start=(j == 0),
                                        stop=(j == NT - 1),
                                    )
                                emit_next(2)
                            for parity in range(2):
                                h = 2 * pair + parity
                                p0 = 64 * parity
                                osl = Onorm[pair][p0 : p0 + 64, cs]
                                nc.vector.tensor_copy(osl, po[parity][0:HD, :])
                                hc = h * 2 + cpass
                                srow = srowpool.tile([1, 512], F32, tag="srow")
                                nc.vector.tensor_copy(
                                    srow[:], po[parity][HD : HD + 1, :]
                                )
                                nc.sync.dma_start(
                                    sums_dram[hc : hc + 1, :], srow[:]
                                )
                        cur_qts = next_qts
                        if pair % 2 == 1:
                            recip_stage(pair // 2)
                        if pair >= 2:
                            mul_pair(pair - 2)
                    for pr in (6, 7):
                        mul_pair(pr)

                # ---- phase D: output projection + bias ------------------
                # ct-outer so only the last 8 matmuls wait on the final
                # pair's normalization; i processed in halves of 4 (8 banks)
                with (
                    tc.tile_pool(name="osb", bufs=2) as osbpool,
                    tc.tile_pool(name="dp", bufs=8, space="PSUM") as dppool,
                ):
                    for half in range(2):
                        i0 = 4 * half
                        pd = [
                            dppool.tile([128, 512], F32, tag="dp", name=f"dp{k}")
                            for k in range(8)
                        ]
                        for ct in range(DT):
                            for di in range(4):
                                i = i0 + di
                                for ec in range(NC2):
                                    nc.tensor.matmul(
                                        pd[2 * di + ec][:],
                                        Onorm[ct][:, i * 128 : (i + 1) * 128],
                                        wp[ct][:, ec * 512 : (ec + 1) * 512],
                                        start=(ct == 0),
                                        stop=False,
                                    )
                        for di in range(4):
                            i = i0 + di
                            ot = osbpool.tile([128, D], F32, tag="osb")
                            for ec in range(NC2):
                                nc.tensor.matmul(
                                    pd[2 * di + ec][:],
                                    ones1[:],
                                    b_sb[:, ec * 512 : (ec + 1) * 512],
                                    start=False,
                                    stop=True,
                                )
                                nc.vector.tensor_copy(
                                    ot[:, ec * 512 : (ec + 1) * 512],
                                    pd[2 * di + ec][:],
                                )
                            nc.sync.dma_start(
                                out[i * 128 : (i + 1) * 128, :], ot[:]
                            )

    nc.compile()
    return nc


_NC = [None]


def _get_nc():
    if _NC[0] is None:
        _NC[0] = build()
    return _NC[0]


def kernel(x, w_qkv, w_proj, b_proj):
    x = np.asarray(x, dtype=np.float32)
    w_qkv = np.asarray(w_qkv, dtype=np.float32)
    w_proj = np.asarray(w_proj, dtype=np.float32)
    b_proj = np.asarray(b_proj, dtype=np.float32)
    assert x.shape == (B, N, D)

    nc = _get_nc()
    in_maps = [
        {"x": x[i], "w_qkv": w_qkv, "w_proj": w_proj, "b_proj": b_proj}
        for i in range(B)
    ]
    trace = os.environ.get("KERNEL_TRACE") == "1"
    res = run_bass_kernel_spmd(
        nc, in_maps, core_ids=list(range(B)), trace=trace
    )
    LAST_EXEC_NS[0] = res.exec_time_ns
    return np.stack([res.results[i]["out"] for i in range(B)], axis=0)
